# revision 1
# baseline (speedup 1.0000x reference)
"""Trainium2 Bass kernel for causal self-attention with RoPE.

Problem: B=2, T=2048, C=1024, H=16 heads, hd=64, fp32 in/out, causal, rotary.

Sharding: 8 cores = 2 batches x 4 head-groups. Core c handles batch c//4 and
heads [4*(c%4), 4*(c%4)+4). Each core computes its heads' Q/K/V projections,
RoPE, causal attention, and a partial output projection over its 256 input
channels; the host sums the 4 partial projections per batch and adds bp.

Key scheduling ideas (everything is ~512-col units, one PSUM bank each, with
manual bank assignment over the 8 banks):
  A: QK projections+rope for t<1024 and V' tiles s<1024.
  B: w0/w1 attention (2-head round-robin per pair) with the QK-h1 / V' 8..15
     units injected between rounds as PE filler.
  C: w2/w3 attention in 4-head round-robin with output-projection units
     (t<1024) as filler.
  D: tail projections for t in [1536,2048).
attV lags scores by 2 rounds so exp (ACT) and the normalization chains are
off the PE critical path. The RoPE half-rotation is a DVE stream_shuffle
(channel pairs packed 16 apart inside each 32-partition quadrant). All matmul
operands are fp16; output is fp16 and the host accumulates in fp32.
"""

import time
from collections import deque
from contextlib import ExitStack

import numpy as np

import concourse.bass as bass
import concourse.tile as tile
from concourse import bacc, library_config, mybir
from concourse.bass_utils import run_bass_kernel_spmd

F32 = mybir.dt.float32
F16 = mybir.dt.float16

T = 2048
C = 1024
HD = 64
NCORES = 8
NEG = -1e10
LAG = 2
SWAP_MASK = list(range(16, 32)) + list(range(16))

AF = mybir.ActivationFunctionType
ALU = mybir.AluOpType

LAST_EXEC_NS = None
LAST_RESULTS = None


def build_nc():
    nc = bacc.Bacc("TRN2", target_bir_lowering=False, debug=False)

    xT = nc.dram_tensor("xT", [C + 1, T], F16, kind="ExternalInput").ap()
    wqT = nc.dram_tensor("wqT", [C, 256], F16, kind="ExternalInput").ap()
    wkT = nc.dram_tensor("wkT", [C, 256], F16, kind="ExternalInput").ap()
    wvT = nc.dram_tensor("wvT", [C + 128, 256], F16, kind="ExternalInput").ap()
    wpT = nc.dram_tensor("wpT", [256, C], F16, kind="ExternalInput").ap()
    bqk = nc.dram_tensor("bqk", [128, 4], F32, kind="ExternalInput").ap()
    cc_d = nc.dram_tensor("cc", [128, T], F16, kind="ExternalInput").ap()
    ss_d = nc.dram_tensor("ss", [128, T], F16, kind="ExternalInput").ap()
    tri_d = nc.dram_tensor("tri", [128, 128], F32, kind="ExternalInput").ap()
    out_d = nc.dram_tensor("out", [T, C], F16, kind="ExternalOutput").ap()

    with tile.TileContext(nc) as tc, ExitStack() as ctx:
        consts = ctx.enter_context(tc.tile_pool(name="consts", bufs=1))
        nc.gpsimd.load_library(library_config.attn)

        cc_sb = consts.tile([128, T], F16)
        ss_sb = consts.tile([128, T], F16)
        tri_sb = consts.tile([128, 128], F32)
        bqk_sb = consts.tile([128, 4], F32)
        x1 = consts.tile([1, T], F16)

        # rotated Q^T / K^T: [pair][half] tiles (Q pairs 0-1, K pairs 2-3)
        qkt = [[consts.tile([128, 1024], F16, name=f"qkt{p}_{h}")
                for h in range(2)] for p in range(4)]
        vp = [consts.tile([128, 4 * 65], F16, name=f"vp{i}") for i in range(16)]
        vview = [v.rearrange("p (h d) -> p h d", d=65) for v in vp]
        usc = [[consts.tile([128, 512], F16, name=f"usc{p}_{w}")
                for w in range(4)] for p in range(2)]
        wp_sb = [consts.tile([128, C], F16, name=f"wp{p}") for p in range(2)]
        xts = [consts.tile([128, T], F16, name=f"xt{j}") for j in range(8)]
        wq_sb = [consts.tile([128, 256], F16, name=f"wq{j}") for j in range(8)]
        wk_sb = [consts.tile([128, 256], F16, name=f"wk{j}") for j in range(8)]
        wv_sb = [consts.tile([128, 256], F16, name=f"wv{j}") for j in range(9)]
        wv1 = wv_sb[8]

        h0, h1 = slice(0, 1024), slice(1024, 2048)

        # ------------- input DMA: priority order, spread over queues -------
        nc.sync.dma_start(cc_sb[:, 0:512], cc_d[:, 0:512])
        nc.sync.dma_start(ss_sb[:, 0:512], ss_d[:, 0:512])
        for j in range(8):
            nc.sync.dma_start(wq_sb[j][:], wqT[128 * j:128 * (j + 1), :])
            nc.sync.dma_start(xts[j][:, h0], xT[128 * j:128 * (j + 1), h0])
        nc.sync.dma_start(bqk_sb[:], bqk[:])
        nc.sync.dma_start(cc_sb[:, 512:1024], cc_d[:, 512:1024])
        nc.sync.dma_start(ss_sb[:, 512:1024], ss_d[:, 512:1024])
        for j in range(8):
            nc.sync.dma_start(wk_sb[j][:], wkT[128 * j:128 * (j + 1), :])
            nc.sync.dma_start(xts[j][:, h1], xT[128 * j:128 * (j + 1), h1])
        for j in range(9):
            nc.sync.dma_start(wv_sb[j][:], wvT[128 * j:128 * (j + 1), :])
        nc.sync.dma_start(tri_sb[:], tri_d[:])
        for q in range(2, 4):
            nc.sync.dma_start(cc_sb[:, 512 * q:512 * (q + 1)],
                              cc_d[:, 512 * q:512 * (q + 1)])
            nc.sync.dma_start(ss_sb[:, 512 * q:512 * (q + 1)],
                              ss_d[:, 512 * q:512 * (q + 1)])
        for p in range(2):
            nc.sync.dma_start(wp_sb[p][:], wpT[128 * p:128 * (p + 1), :])

        nc.gpsimd.memset(x1[:], 1.0)
        for i in range(16):
            nc.gpsimd.memset(vview[i][:, :, 64], 1.0)

        # persistent SBUF pools
        rp = ctx.enter_context(tc.tile_pool(name="rope", bufs=2))
        epl = ctx.enter_context(tc.tile_pool(name="epool", bufs=16))
        zrp = ctx.enter_context(tc.tile_pool(name="zrpool", bufs=4))
        rzbp = ctx.enter_context(tc.tile_pool(name="rzbpool", bufs=4))
        ost = ctx.enter_context(tc.tile_pool(name="ostage", bufs=4))
        # one PSUM pool, manual bank assignment via tags b0..b7
        pb = ctx.enter_context(tc.tile_pool(name="pbank", bufs=1,
                                            space="PSUM"))

        def bank(k, name):
            return pb.tile([128, 512], F32, tag=f"b{k}", name=name)

        mm_ctr = [0]
        s_ctr = [0]

        # ---------------- unit definitions ----------------
        def qk_unit(wsb, ci, half, tg):
            """One 512-col projection+rope unit for Q/K chunk ci."""
            csl = slice(1024 * half + 512 * tg, 1024 * half + 512 * tg + 512)
            wsl = slice(512 * tg, 512 * tg + 512)
            isk = wsb is wk_sb
            ps = bank(mm_ctr[0] % 2, f"qk{isk}_{ci}_{half}_{tg}")
            mm_ctr[0] += 1
            for j in range(8):
                nc.tensor.matmul(ps[:], wsb[j][:, 128 * ci:128 * (ci + 1)],
                                 xts[j][:, csl], start=(j == 0), stop=(j == 7))
            bcol = (2 if isk else 0) + ci
            pair = (2 if isk else 0) + ci
            bias = bqk_sb[:, bcol:bcol + 1]
            p1 = rp.tile([128, 512], F16, tag="p1")
            p2 = rp.tile([128, 512], F16, tag="p2")
            p2s = rp.tile([128, 512], F16, tag="p2s")
            nc.vector.scalar_tensor_tensor(
                out=p1[:], in0=ps[:], scalar=bias, in1=cc_sb[:, csl],
                op0=ALU.add, op1=ALU.mult)
            nc.vector.scalar_tensor_tensor(
                out=p2[:], in0=ps[:], scalar=bias, in1=ss_sb[:, csl],
                op0=ALU.add, op1=ALU.mult)
            nc.vector.stream_shuffle(p2s[:], p2[:], SWAP_MASK)
            nc.vector.tensor_add(qkt[pair][half][:, wsl], p1[:], p2s[:])

        def v_unit(i):
            """V' s-tile i: vraw = x_i^T @ Wv (+bias), copy into vp[i]."""
            ps = bank(mm_ctr[0] % 2, f"v{i}")
            mm_ctr[0] += 1
            tsl = slice(128 * i, 128 * (i + 1))
            for j in range(8):
                nc.tensor.matmul(ps[:, 0:256], xts[j][:, tsl], wv_sb[j][:],
                                 start=(j == 0), stop=False)
            nc.tensor.matmul(ps[:, 0:256], x1[:, tsl], wv1[0:1, :],
                             start=False, stop=True)
            nc.scalar.activation(vview[i][:, :, 0:64], ps[:, 0:256], AF.Copy)

        yz_live = {}
        st_live = {}

        def score_unit(pr, hs, w, i, nbanks, bank0, pool=None):
            """Scores for head (pr,hs), window w, s-tile i; exp to an e-tile."""
            h = 2 * pr + hs
            rows = slice(64 * hs, 64 * (hs + 1))
            sub0 = max(0, 128 * i - 512 * w)
            kt = qkt[2 + pr][i // 8]
            qt = qkt[pr][w // 2]
            qsl = slice((512 * w) % 1024 + sub0, (512 * w) % 1024 + 512)
            s_ps = bank(bank0 + s_ctr[0] % nbanks, f"s{h}_{w}_{i}")
            s_ctr[0] += 1
            nc.tensor.matmul(
                s_ps[:, sub0:512],
                kt[rows, 128 * (i % 8):128 * (i % 8) + 128],
                qt[rows, qsl], start=True, stop=True)
            if i >= 4 * w:
                nc.vector.tensor_add(
                    s_ps[:, sub0:sub0 + 128], s_ps[:, sub0:sub0 + 128],
                    tri_sb[:])
            et = (pool or epl).tile([128, 512], F16, tag="e",
                                    name=f"e{h}_{w}_{i}")
            nc.scalar.activation(et[:, sub0:512], s_ps[:, sub0:512],
                                 AF.Exp, scale=0.125)
            return et, sub0

        def attv_unit(pr, hs, w, i, et, sub0, ni):
            h = 2 * pr + hs
            if i == 0:
                yz_live[h] = bank(4 + h, f"yz{h}_{w}")
            yz = yz_live[h]
            nc.tensor.matmul(yz[0:65, sub0:512],
                             vp[i][:, 65 * h:65 * (h + 1)],
                             et[:, sub0:512], start=(i == 0),
                             stop=(i == ni - 1))

        def norm_unit(pr, hs, w):
            h = 2 * pr + hs
            yz = yz_live[h]
            zrow = zrp.tile([1, 512], F32, tag="zrow", name=f"zr{h}_{w}")
            nc.vector.tensor_copy(zrow[:], yz[64:65, :])
            rzr = zrp.tile([1, 512], F32, tag="rzr", name=f"rr{h}_{w}")
            nc.vector.reciprocal_approx_fast(rzr[:], zrow[:])
            rzb = rzbp.tile([64, 512], F32, tag="rzb", name=f"rb{h}_{w}")
            nc.gpsimd.partition_broadcast(rzb[:], rzr[:])
            nc.vector.tensor_mul(
                usc[pr][w][64 * hs:64 * (hs + 1), :], yz[0:64, :], rzb[:])

        def proj_unit(tch, cg, bk=0):
            w = tch // 4
            tsl = slice(128 * (tch % 4), 128 * (tch % 4) + 128)
            csl = slice(512 * cg, 512 * (cg + 1))
            ps = bank(bk, f"op{tch}_{cg}")
            for pq in range(2):
                nc.tensor.matmul(ps[:], usc[pq][w][:, tsl],
                                 wp_sb[pq][:, csl],
                                 start=(pq == 0), stop=(pq == 1))
            if cg == 0:
                st_live[tch] = ost.tile([128, 1024], F16, tag="ost",
                                        name=f"st{tch}")
            st = st_live[tch]
            nc.vector.tensor_copy(st[:, csl], ps[:])
            if cg == 1:
                nc.sync.dma_start(out_d[128 * tch:128 * tch + 128, :], st[:])

        def window_rounds(w, heads, yq, s_nbanks, s_bank0):
            """Attention window w for the given heads, attV lagging LAG
            rounds behind scores; filler units popped between the two."""
            ni = 4 * w + 4
            nr = ni + LAG
            pend = {hh: deque() for hh in heads}
            ny0 = len(yq)
            emitted = 0
            for r in range(nr):
                if r < ni:
                    for hh in heads:
                        pend[hh].append(
                            (r, *score_unit(*hh, w, r, s_nbanks, s_bank0)))
                while yq and emitted < (r + 1) * ny0 // nr:
                    yq.popleft()()
                    emitted += 1
                if r >= LAG:
                    for hh in heads:
                        i, et, sub0 = pend[hh].popleft()
                        attv_unit(*hh, w, i, et, sub0, ni)
            for hh in heads:
                norm_unit(*hh, w)

        # ---------------- A: QK h0 + V' 0..7 ----------------
        aunits = []
        for ci in range(2):
            for tg in range(2):
                aunits.append(lambda ci=ci, tg=tg: qk_unit(wq_sb, ci, 0, tg))
        for ci in range(2):
            for tg in range(2):
                aunits.append(lambda ci=ci, tg=tg: qk_unit(wk_sb, ci, 0, tg))
        for i in range(8):
            aunits.append(lambda i=i: v_unit(i))
        for u in aunits:
            u()

        # ---------------- B: w0/w1 attention + h1/V filler ----------------
        yq = deque()
        for ci in range(2):
            for tg in range(2):
                yq.append(lambda ci=ci, tg=tg: qk_unit(wq_sb, ci, 1, tg))
                yq.append(lambda ci=ci, tg=tg: qk_unit(wk_sb, ci, 1, tg))
        for i in range(8, 16):
            yq.append(lambda i=i: v_unit(i))
        nyb = len(yq)
        # hand out filler roughly evenly across the four pair-windows
        shares = [nyb * 6 // 32, nyb * 12 // 32, nyb * 22 // 32, nyb]
        prev = 0
        for wi, (w, pr) in enumerate([(0, 0), (0, 1), (1, 0), (1, 1)]):
            cnt = shares[wi] - prev
            prev = shares[wi]
            sub = deque(yq.popleft() for _ in range(cnt))
            window_rounds(w, [(pr, 0), (pr, 1)], sub, 2, 2)
            while sub:
                sub.popleft()()

        # ---------------- C: w2 attention + proj filler + w3 precompute ---
        # All of w3's scores+exp run as filler inside the w2 window, into a
        # deep SBUF e-buffer; the w3 window is then pure attV on the PE.
        ep2 = ctx.enter_context(tc.tile_pool(name="ep2", bufs=56))
        pre_e = {}

        def w3s(pr, hs, i):
            pool = ep2 if i < 14 else epl
            pre_e[(2 * pr + hs, i)] = score_unit(pr, hs, 3, i, 3, 1,
                                                 pool=pool)

        yq = deque()
        projq = deque()
        for tch in range(8):
            for cg in range(2):
                projq.append(lambda tch=tch, cg=cg: proj_unit(tch, cg))
        for i in range(14):
            if projq:
                yq.append(projq.popleft())
            for pr in range(2):
                for hs in range(2):
                    yq.append(lambda pr=pr, hs=hs, i=i: w3s(pr, hs, i))
        yq.extend(projq)
        window_rounds(2, [(0, 0), (0, 1), (1, 0), (1, 1)], yq, 3, 1)
        while yq:
            yq.popleft()()
        # w3: attV-only rounds from precomputed e-tiles + tch8-11 proj
        yq = deque()
        for tch in range(8, 12):
            for cg in range(2):
                yq.append(lambda tch=tch, cg=cg:
                          proj_unit(tch, cg, bk=(2 * tch + cg) % 2))
        for i in range(14, 16):
            for pr in range(2):
                for hs in range(2):
                    w3s(pr, hs, i)
        emitted = 0
        for r in range(16):
            for pr in range(2):
                for hs in range(2):
                    et, sub0 = pre_e[(2 * pr + hs, r)]
                    attv_unit(pr, hs, 3, r, et, sub0, 16)
            while yq and emitted < (r + 1) * 8 // 16:
                yq.popleft()()
                emitted += 1
        while yq:
            yq.popleft()()
        for pr in range(2):
            for hs in range(2):
                norm_unit(pr, hs, 3)

        # ---------------- D: tail projections ----------------
        for tch in range(12, 16):
            for cg in range(2):
                proj_unit(tch, cg, bk=(2 * tch + cg) % 2)

    nc.compile()
    return nc


_NC_CACHE = {}


def _get_nc():
    if "nc" not in _NC_CACHE:
        _NC_CACHE["nc"] = build_nc()
    return _NC_CACHE["nc"]


def make_in_map(core, x, Wq, bq, Wk, bk, Wv, bv, Wp, bp, rope_cache):
    b = core // 4
    hbase = (core % 4) * 4

    xTa = np.empty((C + 1, T), np.float16)
    xTa[:C] = np.asarray(x[b], np.float32).T
    xTa[C] = 1.0

    # packed channel order for Q/K: per head, two 32-row quadrants; each
    # quadrant holds [even ch 16q..16q+15 | odd ch 16q..16q+15] so the rope
    # partner swap is lane l -> (l+16)%32 inside every quadrant.
    perm = []
    for p in range(2):
        for hh in range(2):
            h = hbase + 2 * p + hh
            for q in range(2):
                perm += [h * HD + 2 * (16 * q + m) for m in range(16)]
                perm += [h * HD + 2 * (16 * q + m) + 1 for m in range(16)]
    perm = np.asarray(perm)

    wqTa = np.ascontiguousarray(
        np.asarray(Wq, np.float32)[perm, :].T).astype(np.float16)
    wkTa = np.ascontiguousarray(
        np.asarray(Wk, np.float32)[perm, :].T).astype(np.float16)

    chs = np.arange(hbase * HD, hbase * HD + 256)
    wvTa = np.zeros((C + 128, 256), np.float16)
    wvTa[:C] = np.asarray(Wv, np.float32)[chs, :].T
    wvTa[C] = np.asarray(bv, np.float32)[chs]
    wpTa = np.ascontiguousarray(
        np.asarray(Wp, np.float32)[:, chs].T).astype(np.float16)

    bqp = np.asarray(bq, np.float32)[perm].reshape(2, 128).T
    bkp = np.asarray(bk, np.float32)[perm].reshape(2, 128).T
    bqk_a = np.concatenate([bqp, bkp], axis=1)  # [128, 4]

    rc = np.asarray(rope_cache, np.float32)  # [T, 32, 2]
    r = np.arange(128)
    lane = r % 32
    quad = (r // 32) % 2
    m = 16 * quad + (lane % 16)  # rotation pair index per row
    sign = np.where(lane < 16, 1.0, -1.0).astype(np.float32)
    cc_a = np.ascontiguousarray(rc[:, m, 0].T).astype(np.float16)
    ss_a = np.ascontiguousarray(
        (rc[:, m, 1].T * sign[:, None])).astype(np.float16)

    sl, tl = np.arange(128)[:, None], np.arange(128)[None, :]
    tri_a = np.where(tl >= sl, 0.0, NEG).astype(np.float32)

    return dict(xT=xTa, wqT=wqTa, wkT=wkTa, wvT=wvTa, wpT=wpTa,
                bqk=bqk_a, cc=cc_a, ss=ss_a, tri=tri_a)


def kernel(x, Wq, bq, Wk, bk, Wv, bv, Wp, bp, rope_cache):
    global LAST_EXEC_NS, LAST_RESULTS
    args = (x, Wq, bq, Wk, bk, Wv, bv, Wp, bp, rope_cache)
    nc = _get_nc()
    in_maps = [make_in_map(c, *args) for c in range(NCORES)]
    r = None
    for attempt in range(4):
        try:
            r = run_bass_kernel_spmd(nc, in_maps, list(range(NCORES)))
            break
        except Exception:
            # transient NRT exec-unit errors recover on re-dispatch
            if attempt == 3:
                raise
            time.sleep(5.0 * (attempt + 1))
    LAST_EXEC_NS = r.exec_time_ns
    LAST_RESULTS = r
    out = np.zeros((2, T, C), np.float32)
    for core in range(NCORES):
        out[core // 4] += np.asarray(r.results[core]["out"], np.float32)
    out += np.asarray(bp, np.float32)[None, None, :]
    return out



# revision 10
# speedup vs baseline: 1.0918x; 1.0918x over previous
"""Trainium2 Bass kernel for causal self-attention with RoPE.

Problem: B=2, T=2048, C=1024, H=16 heads, hd=64, fp32 in/out, causal, rotary.

Sharding: 8 cores = 2 batches x 4 head-groups. Core c handles batch c//4 and
heads [4*(c%4), 4*(c%4)+4). Each core computes its heads' Q/K/V projections,
RoPE, causal attention, and a partial output projection over its 256 input
channels; the host sums the 4 partial projections per batch and adds bp.

v2 design notes (from the v1 trace: ACT/exp is the co-bottleneck with PE,
and the v1 tail serialized on norm chains):
 - The two heads of a pair write adjacent PSUM banks of one [128,1024] tile
   and a single EXP covers both (1147ns vs 2x720ns); deep-diagonal rounds
   use two trimmed exps instead.
 - PSUM tags: pA/pB = two [128,1024] score tiles (2 banks each), y0..y3 =
   per-head attV accumulators (64 y-rows + z-row). Attention windows run
   per-pair so the inactive pair's y-banks serve as filler/proj PSUM.
 - Rope: ACT does the bias-add + fp32->fp16 cast out of PSUM (Identity with
   per-partition bias); sin is pre-shuffled on the host so DVE does only
   mul/shuffle/mul/add on fp16.
 - All output projections run as PE filler inside the ACT-bound score phases
   or the attV-w3 phase; out-DMA goes per 512-col chunk after each cast.
 - Input DMA: few big multi-dim transfers (v1 startup was sync-issue bound),
   ordered so V-units (cheapest deps) start first; issued from 4 engines.
 - Norm chains: reciprocal reads z directly from PSUM, gpsimd broadcasts,
   DVE multiplies; per-head chains are emitted stage-interleaved.
"""

import time
from collections import deque
from contextlib import ExitStack

import numpy as np

import concourse.bass as bass
import concourse.tile as tile
from concourse import bacc, library_config, mybir
from concourse.bass_utils import run_bass_kernel_spmd

F32 = mybir.dt.float32
F16 = mybir.dt.float16

T = 2048
C = 1024
HD = 64
NCORES = 8
NEG = -1e10
LAG = 2
SWAP_MASK = list(range(16, 32)) + list(range(16))

AF = mybir.ActivationFunctionType
ALU = mybir.AluOpType

LAST_EXEC_NS = None
LAST_RESULTS = None


def build_nc():
    nc = bacc.Bacc("TRN2", target_bir_lowering=False, debug=False)

    xT = nc.dram_tensor("xT", [C + 1, T], F16, kind="ExternalInput").ap()
    wqT = nc.dram_tensor("wqT", [C, 256], F16, kind="ExternalInput").ap()
    wkT = nc.dram_tensor("wkT", [C, 256], F16, kind="ExternalInput").ap()
    wvT = nc.dram_tensor("wvT", [C + 128, 256], F16, kind="ExternalInput").ap()
    wpT = nc.dram_tensor("wpT", [256, C], F16, kind="ExternalInput").ap()
    bqk = nc.dram_tensor("bqk", [128, 4], F32, kind="ExternalInput").ap()
    cc_d = nc.dram_tensor("cc", [128, T], F16, kind="ExternalInput").ap()
    ss_d = nc.dram_tensor("ss", [128, T], F16, kind="ExternalInput").ap()
    tri_d = nc.dram_tensor("tri", [128, 128], F32, kind="ExternalInput").ap()
    out_d = nc.dram_tensor("out", [T, C], F16, kind="ExternalOutput").ap()

    with tile.TileContext(nc) as tc, ExitStack() as ctx:
        consts = ctx.enter_context(tc.tile_pool(name="consts", bufs=1))
        nc.gpsimd.load_library(library_config.attn)

        cc_sb = consts.tile([128, T], F16)
        ss_sb = consts.tile([128, T], F16)   # pre-shuffled+signed sin
        tri_sb = consts.tile([128, 128], F32)
        bqk_sb = consts.tile([128, 4], F32)
        x1 = consts.tile([1, T], F16)

        # rotated Q^T / K^T: [pair][half] tiles (Q pairs 0-1, K pairs 2-3)
        qkt = [[consts.tile([128, 1024], F16, name=f"qkt{p}_{h}")
                for h in range(2)] for p in range(4)]
        vp = [consts.tile([128, 4 * 65], F16, name=f"vp{i}") for i in range(16)]
        vview = [v.rearrange("p (h d) -> p h d", d=65) for v in vp]
        usc = [[consts.tile([128, 512], F16, name=f"usc{p}_{w}")
                for w in range(4)] for p in range(2)]
        wp_sb = [consts.tile([128, C], F16, name=f"wp{p}") for p in range(2)]
        xts = [consts.tile([128, T], F16, name=f"xt{j}") for j in range(8)]
        # packed weights: 8 (9 for V) row-chunks side by side in the free dim
        wq_sb = consts.tile([128, 8 * 256], F16)
        wk_sb = consts.tile([128, 8 * 256], F16)
        wv_sb = consts.tile([128, 9 * 256], F16)

        h0, h1 = slice(0, 1024), slice(1024, 2048)

        # ---------- input DMA: few big transfers, priority order ----------
        wv_src = wvT.rearrange("(a p) c -> p a c", p=128)
        wq_src = wqT.rearrange("(a p) c -> p a c", p=128)
        wk_src = wkT.rearrange("(a p) c -> p a c", p=128)
        wp_src = wpT.rearrange("(a p) c -> p a c", p=128)

        nc.sync.dma_start(wv_sb.rearrange("p (a c) -> p a c", c=256), wv_src)
        nc.gpsimd.dma_start(bqk_sb[:], bqk[:])
        nc.gpsimd.dma_start(cc_sb[:, h0], cc_d[:, h0])
        nc.gpsimd.dma_start(ss_sb[:, h0], ss_d[:, h0])
        nc.scalar.dma_start(wq_sb.rearrange("p (a c) -> p a c", c=256),
                            wq_src)
        for j in range(4):
            nc.sync.dma_start(xts[j][:, h0], xT[128 * j:128 * (j + 1), h0])
        nc.scalar.dma_start(tri_sb[:], tri_d[:])
        for j in range(4, 8):
            nc.sync.dma_start(xts[j][:, h0], xT[128 * j:128 * (j + 1), h0])
        nc.scalar.dma_start(wk_sb.rearrange("p (a c) -> p a c", c=256),
                            wk_src)
        for j in range(4):
            nc.sync.dma_start(xts[j][:, h1], xT[128 * j:128 * (j + 1), h1])
        nc.gpsimd.dma_start(cc_sb[:, h1], cc_d[:, h1])
        nc.gpsimd.dma_start(ss_sb[:, h1], ss_d[:, h1])
        for j in range(4, 8):
            nc.sync.dma_start(xts[j][:, h1], xT[128 * j:128 * (j + 1), h1])
        nc.scalar.dma_start(wp_sb[0][:], wp_src[:, 0, :])
        nc.scalar.dma_start(wp_sb[1][:], wp_src[:, 1, :])

        nc.gpsimd.memset(x1[:], 1.0)
        for i in range(16):
            nc.gpsimd.memset(vview[i][:, :, 64], 1.0)

        # persistent SBUF pools
        rp = ctx.enter_context(tc.tile_pool(name="rope", bufs=3))
        epl = ctx.enter_context(tc.tile_pool(name="epool", bufs=5))
        ep2 = ctx.enter_context(tc.tile_pool(name="ep2", bufs=18))
        zrp = ctx.enter_context(tc.tile_pool(name="zrpool", bufs=4))
        rzbp = ctx.enter_context(tc.tile_pool(name="rzbpool", bufs=4))
        ost = ctx.enter_context(tc.tile_pool(name="ostage", bufs=6))
        pb = ctx.enter_context(tc.tile_pool(name="pbank", bufs=1,
                                            space="PSUM"))

        # PSUM: pA/pB two-bank score tiles, y0..y3 single-bank accumulators
        def pair_bank(tag, name):
            return pb.tile([128, 1024], F32, tag=tag, name=name)

        def ybank(k, name):
            return pb.tile([128, 512], F32, tag=f"y{k}", name=name)

        # filler psum: rotate over the y-banks listed in fb_state
        fb_state = {"banks": (0, 1, 2, 3), "ctr": 0}

        def fslot(name):
            banks = fb_state["banks"]
            k = banks[fb_state["ctr"] % len(banks)]
            fb_state["ctr"] += 1
            return ybank(k, name)

        # ---------------- unit definitions ----------------
        def qk_unit(isk, ci, half, tg):
            """One 512-col projection+rope unit for Q/K chunk ci."""
            csl = slice(1024 * half + 512 * tg, 1024 * half + 512 * tg + 512)
            wsl = slice(512 * tg, 512 * tg + 512)
            wsb = wk_sb if isk else wq_sb
            ps = fslot(f"qk{int(isk)}_{ci}_{half}_{tg}")
            for j in range(8):
                nc.tensor.matmul(
                    ps[:], wsb[:, 256 * j + 128 * ci:256 * j + 128 * ci + 128],
                    xts[j][:, csl], start=(j == 0), stop=(j == 7))
            bcol = (2 if isk else 0) + ci
            pair = (2 if isk else 0) + ci
            bias = bqk_sb[:, bcol:bcol + 1]
            pb16 = rp.tile([128, 512], F16, tag="pb16")
            t1 = rp.tile([128, 512], F16, tag="t1")
            shf = rp.tile([128, 512], F16, tag="shf")
            t2 = rp.tile([128, 512], F16, tag="t2")
            nc.scalar.activation(pb16[:], ps[:], AF.Identity, bias=bias)
            nc.vector.tensor_mul(t1[:], pb16[:], cc_sb[:, csl])
            nc.vector.stream_shuffle(shf[:], pb16[:], SWAP_MASK)
            nc.vector.tensor_mul(t2[:], shf[:], ss_sb[:, csl])
            nc.vector.tensor_add(qkt[pair][half][:, wsl], t1[:], t2[:])

        def v_unit(i):
            """V' s-tile i: vraw = x_i^T @ Wv (+bias), copy into vp[i]."""
            ps = fslot(f"v{i}")
            tsl = slice(128 * i, 128 * (i + 1))
            for j in range(8):
                nc.tensor.matmul(ps[:, 0:256], xts[j][:, tsl],
                                 wv_sb[:, 256 * j:256 * (j + 1)],
                                 start=(j == 0), stop=False)
            nc.tensor.matmul(ps[:, 0:256], x1[:, tsl],
                             wv_sb[0:1, 2048:2304], start=False, stop=True)
            nc.scalar.activation(vview[i][:, :, 0:64], ps[:, 0:256], AF.Copy)

        yz_live = {}

        def score_round(pr, w, i, tag, pool):
            """Merged score round: both heads of pair pr, s-tile i, window w.
            Both heads land in the two banks of one [128,1024] tile; one
            merged exp (or two trimmed ones on deep-diagonal rounds)."""
            sub0 = max(0, 128 * i - 512 * w)
            kt = qkt[2 + pr][i // 8]
            qt = qkt[pr][w // 2]
            qsl = slice((512 * w) % 1024 + sub0, (512 * w) % 1024 + 512)
            ps = pair_bank(tag, f"s{pr}_{w}_{i}")
            for hs in range(2):
                rows = slice(64 * hs, 64 * (hs + 1))
                nc.tensor.matmul(
                    ps[:, 512 * hs + sub0:512 * (hs + 1)],
                    kt[rows, 128 * (i % 8):128 * (i % 8) + 128],
                    qt[rows, qsl], start=True, stop=True)
            if i >= 4 * w:
                for hs in range(2):
                    o = 512 * hs + sub0
                    nc.vector.tensor_add(ps[:, o:o + 128], ps[:, o:o + 128],
                                         tri_sb[:])
            et = pool.tile([128, 1024], F16, tag="e", name=f"e{pr}_{w}_{i}")
            if sub0 > 0:
                for hs in range(2):
                    o = 512 * hs + sub0
                    e = 512 * (hs + 1)
                    nc.scalar.activation(et[:, o:e], ps[:, o:e], AF.Exp,
                                         scale=0.125)
            else:
                nc.scalar.activation(et[:], ps[:], AF.Exp, scale=0.125)
            return et, sub0

        def attv_round(pr, w, i, et, sub0, ni):
            for hs in range(2):
                h = 2 * pr + hs
                if i == 0:
                    yz_live[h] = ybank(h, f"yz{h}_{w}")
                nc.tensor.matmul(
                    yz_live[h][0:65, sub0:512],
                    vp[i][:, 65 * h:65 * (h + 1)],
                    et[:, 512 * hs + sub0:512 * (hs + 1)],
                    start=(i == 0), stop=(i == ni - 1))

        def norm_pair(pr, w):
            """Normalize both heads of pair pr for window w into usc."""
            rz, rb = {}, {}
            zr = {}
            for hs in range(2):
                h = 2 * pr + hs
                # PSUM holds e10m23; the recip's bitwise seed needs IEEE fp32
                # bits, so bounce z through SBUF via the (idle) ACT engine.
                zr[hs] = zrp.tile([1, 512], F32, tag="zrow", name=f"zc{h}_{w}")
                nc.scalar.activation(zr[hs][:], yz_live[h][64:65, :], AF.Copy)
            for hs in range(2):
                h = 2 * pr + hs
                rz[hs] = zrp.tile([1, 512], F32, tag="rzr", name=f"rr{h}_{w}")
                nc.vector.reciprocal_approx_fast(rz[hs][:], zr[hs][:])
            for hs in range(2):
                h = 2 * pr + hs
                rb[hs] = rzbp.tile([64, 512], F32, tag="rzb",
                                   name=f"rb{h}_{w}")
                nc.gpsimd.partition_broadcast(rb[hs][:], rz[hs][:])
            for hs in range(2):
                h = 2 * pr + hs
                nc.vector.tensor_mul(usc[pr][w][64 * hs:64 * (hs + 1), :],
                                     yz_live[h][0:64, :], rb[hs][:])

        def proj_unit(tch, cg, psl, on_act=False):
            """Output projection for t-chunk tch, 512-col group cg."""
            w = tch // 4
            tsl = slice(128 * (tch % 4), 128 * (tch % 4) + 128)
            csl = slice(512 * cg, 512 * (cg + 1))
            for pq in range(2):
                nc.tensor.matmul(psl[:, 0:512], usc[pq][w][:, tsl],
                                 wp_sb[pq][:, csl],
                                 start=(pq == 0), stop=(pq == 1))
            st = ost.tile([128, 512], F16, tag="ost", name=f"st{tch}_{cg}")
            if on_act:
                nc.scalar.activation(st[:], psl[:, 0:512], AF.Copy)
            else:
                nc.vector.tensor_copy(st[:], psl[:, 0:512])
            nc.sync.dma_start(out_d[128 * tch:128 * tch + 128, csl], st[:])

        # ---------------- A: startup stream ----------------
        # v0-3 rotate y0..y3; the first two qk units go once more through
        # y0/y1 (their readers are long done before w0's yz claims them).
        fb_state["banks"] = (0, 1, 2, 3)
        for i in range(4):
            v_unit(i)
        qk_unit(False, 0, 0, 0)
        qk_unit(True, 0, 0, 0)

        # ---------------- B: w0/w1 per pair + filler ----------------
        def window(w, pr, fill, nfill_share):
            """Attention window w for pair pr; scores alternate pA/pB,
            attV lags LAG rounds; fillers popped between."""
            ni = 4 * w + 4
            nr = ni + LAG
            pend = deque()
            emitted = 0
            for r in range(nr):
                if r < ni:
                    et, sub0 = score_round(pr, w, r,
                                           "pA" if r % 2 == 0 else "pB", epl)
                    pend.append((r, et, sub0))
                while fill and emitted < (r + 1) * nfill_share // nr:
                    fill.popleft()()
                    emitted += 1
                if r >= LAG:
                    i, et, sub0 = pend.popleft()
                    attv_round(pr, w, i, et, sub0, ni)
            norm_pair(pr, w)

        fill = deque()
        fill.append(lambda: qk_unit(False, 1, 0, 0))
        fill.append(lambda: qk_unit(True, 1, 0, 0))
        for i in range(4, 8):
            fill.append(lambda i=i: v_unit(i))
        for isk in (False, True):
            fill.append(lambda isk=isk: qk_unit(isk, 0, 0, 1))
        for isk in (False, True):
            fill.append(lambda isk=isk: qk_unit(isk, 1, 0, 1))
        for isk in (False, True):
            for tg in range(2):
                fill.append(lambda isk=isk, tg=tg: qk_unit(isk, 0, 1, tg))
        for i in range(8, 12):
            fill.append(lambda i=i: v_unit(i))
        for isk in (False, True):
            for tg in range(2):
                fill.append(lambda isk=isk, tg=tg: qk_unit(isk, 1, 1, tg))
        for i in range(12, 16):
            fill.append(lambda i=i: v_unit(i))

        nf = len(fill)
        shares = [nf * 5 // 32, nf * 12 // 32, nf * 22 // 32, nf]
        prev = 0
        for wi, (w, pr) in enumerate([(0, 0), (0, 1), (1, 0), (1, 1)]):
            # fillers use the y-banks of the inactive pair
            fb_state["banks"] = (2, 3) if pr == 0 else (0, 1)
            cnt = shares[wi] - prev
            prev = shares[wi]
            sub = deque(fill.popleft() for _ in range(cnt))
            window(w, pr, sub, cnt)
            while sub:
                sub.popleft()()

        # ---------------- C: per-pair w2 window + w3 scores ----------------
        pre_e = {}

        def phase_c(pr):
            """w2 window for pair pr + all w3 scores for pair pr (deep
            buffer). Fillers: proj tch0-7 (pr=0) / attV-w3 of pair 0 (pr=1).
            Tags staggered: w2 on pA/pB by round parity, w3 opposite."""
            ni2, ni3 = 12, 16
            pend = deque()
            fillq = deque()
            if pr == 0:
                for tch in range(8):
                    for cg in range(2):
                        fillq.append(
                            lambda tch=tch, cg=cg:
                            proj_unit(tch, cg, fslot(f"op{tch}_{cg}")))
            nfq = len(fillq)
            for r in range(ni3 + LAG):
                if r < ni2:
                    et, sub0 = score_round(pr, 2, r,
                                           "pA" if r % 2 == 0 else "pB", epl)
                    pend.append((r, et, sub0))
                if r < ni3:
                    pre_e[(pr, r)] = score_round(pr, 3, r,
                                                 "pB" if r % 2 == 0 else "pA",
                                                 ep2)
                if pr == 0:
                    while fillq and len(fillq) > nfq * (ni3 + LAG - 1 - r) \
                            // (ni3 + LAG):
                        fillq.popleft()()
                elif r < ni3:
                    et0, s00 = pre_e[(0, r)]
                    attv_round(0, 3, r, et0, s00, ni3)
                if r >= LAG and pend:
                    i, et, sub0 = pend.popleft()
                    attv_round(pr, 2, i, et, sub0, ni2)
            norm_pair(pr, 2)
            if pr == 1:
                norm_pair(0, 3)

        fb_state["banks"] = (2, 3)   # proj fillers in C-pr0 use y2/y3
        phase_c(0)
        phase_c(1)

        # ---------------- D: attV w3 pair1 + proj tch8-11 ----------------
        # pA/pB are free now; proj units rotate their four bank-halves.
        pr_slots = {"ctr": 0, "cur": {}}

        def pslot(name):
            k = pr_slots["ctr"] % 4
            pr_slots["ctr"] += 1
            tag = "pA" if k < 2 else "pB"
            half = k % 2
            if half == 0:
                pr_slots["cur"][tag] = pair_bank(tag, name)
            return pr_slots["cur"][tag][:, 512 * half:512 * (half + 1)]

        projq = deque()
        for tch in range(8, 12):
            for cg in range(2):
                projq.append(
                    lambda tch=tch, cg=cg:
                    proj_unit(tch, cg, pslot(f"op{tch}_{cg}"), on_act=True))
        npq = len(projq)
        for r in range(16):
            et1, s01 = pre_e[(1, r)]
            attv_round(1, 3, r, et1, s01, 16)
            while projq and len(projq) > npq * (15 - r) // 16:
                projq.popleft()()
        norm_pair(1, 3)

        # ---------------- E: tail projections ----------------
        for tch in range(12, 16):
            for cg in range(2):
                proj_unit(tch, cg, pslot(f"op{tch}_{cg}"), on_act=True)

    nc.compile()
    return nc


_NC_CACHE = {}


def _get_nc():
    if "nc" not in _NC_CACHE:
        _NC_CACHE["nc"] = build_nc()
    return _NC_CACHE["nc"]


def make_in_map(core, x, Wq, bq, Wk, bk, Wv, bv, Wp, bp, rope_cache):
    b = core // 4
    hbase = (core % 4) * 4

    xTa = np.empty((C + 1, T), np.float16)
    xTa[:C] = np.asarray(x[b], np.float32).T
    xTa[C] = 1.0

    # packed channel order for Q/K: per head, two 32-row quadrants; each
    # quadrant holds [even ch 16q..16q+15 | odd ch 16q..16q+15] so the rope
    # partner swap is lane l -> (l+16)%32 inside every quadrant.
    perm = []
    for p in range(2):
        for hh in range(2):
            h = hbase + 2 * p + hh
            for q in range(2):
                perm += [h * HD + 2 * (16 * q + m) for m in range(16)]
                perm += [h * HD + 2 * (16 * q + m) + 1 for m in range(16)]
    perm = np.asarray(perm)

    wqTa = np.ascontiguousarray(
        np.asarray(Wq, np.float32)[perm, :].T).astype(np.float16)
    wkTa = np.ascontiguousarray(
        np.asarray(Wk, np.float32)[perm, :].T).astype(np.float16)

    chs = np.arange(hbase * HD, hbase * HD + 256)
    wvTa = np.zeros((C + 128, 256), np.float16)
    wvTa[:C] = np.asarray(Wv, np.float32)[chs, :].T
    wvTa[C] = np.asarray(bv, np.float32)[chs]
    wpTa = np.ascontiguousarray(
        np.asarray(Wp, np.float32)[:, chs].T).astype(np.float16)

    bqp = np.asarray(bq, np.float32)[perm].reshape(2, 128).T
    bkp = np.asarray(bk, np.float32)[perm].reshape(2, 128).T
    bqk_a = np.concatenate([bqp, bkp], axis=1)  # [128, 4]

    rc = np.asarray(rope_cache, np.float32)  # [T, 32, 2]
    r = np.arange(128)
    lane = r % 32
    quad = (r // 32) % 2
    m = 16 * quad + (lane % 16)  # rotation pair index per row
    sign = np.where(lane < 16, 1.0, -1.0).astype(np.float32)
    cc_a = np.ascontiguousarray(rc[:, m, 0].T).astype(np.float16)
    ss_raw = (rc[:, m, 1].T * sign[:, None]).astype(np.float16)
    # pre-shuffle sin rows so t2 = shuffle(pb) * ss_pre == shuffle(pb * ss)
    swap = np.asarray(SWAP_MASK)
    rows = np.arange(128)
    src = (rows // 32) * 32 + swap[rows % 32]
    ss_a = np.ascontiguousarray(ss_raw[src, :])

    sl, tl = np.arange(128)[:, None], np.arange(128)[None, :]
    tri_a = np.where(tl >= sl, 0.0, NEG).astype(np.float32)

    return dict(xT=xTa, wqT=wqTa, wkT=wkTa, wvT=wvTa, wpT=wpTa,
                bqk=bqk_a, cc=cc_a, ss=ss_a, tri=tri_a)


def kernel(x, Wq, bq, Wk, bk, Wv, bv, Wp, bp, rope_cache):
    global LAST_EXEC_NS, LAST_RESULTS
    args = (x, Wq, bq, Wk, bk, Wv, bv, Wp, bp, rope_cache)
    nc = _get_nc()
    in_maps = [make_in_map(c, *args) for c in range(NCORES)]
    r = None
    for attempt in range(4):
        try:
            r = run_bass_kernel_spmd(nc, in_maps, list(range(NCORES)))
            break
        except Exception:
            # transient NRT exec-unit errors recover on re-dispatch
            if attempt == 3:
                raise
            time.sleep(5.0 * (attempt + 1))
    LAST_EXEC_NS = r.exec_time_ns
    LAST_RESULTS = r
    out = np.zeros((2, T, C), np.float32)
    for core in range(NCORES):
        out[core // 4] += np.asarray(r.results[core]["out"], np.float32)
    out += np.asarray(bp, np.float32)[None, None, :]
    return out
